# revision 10
# baseline (speedup 1.0000x reference)
"""Trainium2 Bass kernel for prefix-mask (ragged sequence) operation.

out[i, j] = images[i, j] if j < position[i] else 0

Sharding: pure row-parallel across 8 NeuronCores (2048 rows each).
Per core: row-tiles of [128, 4096] are fused in pairs into 4 MiB DMAs
(better DMA efficiency); each tile gets one fused DVE op
  out = (iota < pos_per_partition) * images
via scalar_tensor_tensor (in-place). All DMAs issue on the SP HWDGE
ring (measured ~7 us/iter faster than splitting loads/stores across
the SP and ACT rings).
"""

import numpy as np

import concourse.bacc as bacc
import concourse.mybir as mybir
from concourse.tile import TileContext
from concourse.bass_utils import run_bass_kernel_spmd

N, L = 16384, 4096
M = 8               # cores
R = N // M          # 2048 rows per core
P = 128             # partitions
T = R // P          # 16 row-tiles per core
G = 2               # row-tiles fused per DMA (4 MiB per transfer)
U = T // G          # DMA groups

_NC = None


def _build(reps=1):
    nc = bacc.Bacc(None, target_bir_lowering=False)
    img = nc.dram_tensor("images", [R, L], mybir.dt.float32, kind="ExternalInput")
    pos = nc.dram_tensor("pos", [P, T], mybir.dt.float32, kind="ExternalInput")
    out = nc.dram_tensor("out", [R, L], mybir.dt.float32, kind="ExternalOutput")

    # group u holds row-tiles t = u*G + g; partition p maps to row (u*G+g)*P + p
    img_t = img.rearrange("(u g p) l -> u p g l", p=P, g=G)
    out_t = out.rearrange("(u g p) l -> u p g l", p=P, g=G)

    with TileContext(nc) as tc:
        with (
            tc.tile_pool(name="const", bufs=1) as cpool,
            tc.tile_pool(name="work", bufs=4) as wpool,
        ):
            pos_sb = cpool.tile([P, T], mybir.dt.float32, tag="pos")
            nc.sync.dma_start(out=pos_sb[:, :], in_=pos[:, :])

            iota_raw = cpool.tile([P, L], mybir.dt.float32, tag="iota_raw")
            nc.gpsimd.iota(
                iota_raw[:, :],
                [[1, L]],
                channel_multiplier=0,
                allow_small_or_imprecise_dtypes=True,
            )
            # Joiner: one DVE op reading iota_raw AND pos_sb, so every later
            # DVE op inherits those dependencies via program order instead of
            # carrying extra semaphore waits. (x + p) - p == x exactly for
            # the small integers involved.
            iota = cpool.tile([P, L], mybir.dt.float32, tag="iota")
            nc.vector.tensor_scalar(
                out=iota[:, :],
                in0=iota_raw[:, :],
                scalar1=pos_sb[:, 0:1],
                scalar2=pos_sb[:, 0:1],
                op0=mybir.AluOpType.add,
                op1=mybir.AluOpType.subtract,
            )

            for _ in range(reps):
                for u in range(U):
                    tile = wpool.tile([P, G, L], mybir.dt.float32, tag="img")
                    nc.sync.dma_start(out=tile[:, :, :], in_=img_t[u])
                    for g in range(G):
                        t = u * G + g
                        # in-place: tile = (iota < pos[:, t]) * tile
                        nc.vector.scalar_tensor_tensor(
                            out=tile[:, g, :],
                            in0=iota[:, :],
                            scalar=pos_sb[:, t : t + 1],
                            in1=tile[:, g, :],
                            op0=mybir.AluOpType.is_lt,
                            op1=mybir.AluOpType.mult,
                        )
                    nc.sync.dma_start(out=out_t[u], in_=tile[:, :, :])
    nc.compile()
    return nc


def _get_nc():
    global _NC
    if _NC is None:
        _NC = _build()
    return _NC


def _make_in_maps(images, position):
    images = np.ascontiguousarray(np.asarray(images), dtype=np.float32)
    pos = np.asarray(position).astype(np.float32)
    in_maps = []
    for c in range(M):
        sl = slice(c * R, (c + 1) * R)
        # [T, P] -> [P, T] so column t holds the per-partition positions of tile t
        pos_c = np.ascontiguousarray(pos[sl].reshape(T, P).T)
        in_maps.append({"images": images[sl], "pos": pos_c})
    return in_maps


def _run(in_maps, **kwargs):
    return run_bass_kernel_spmd(_get_nc(), in_maps, core_ids=list(range(M)), **kwargs)


def kernel(images, position):
    res = _run(_make_in_maps(images, position))
    return np.concatenate([r["out"] for r in res.results], axis=0)


# revision 11
# speedup vs baseline: 1.5502x; 1.5502x over previous
"""Trainium2 Bass kernel for prefix-mask (ragged sequence) operation.

out[i, j] = images[i, j] if j < position[i] else 0

Sharding: row-parallel across 8 NeuronCores, with rows SORTED by
position host-side and rank-interleaved across cores so that DMA group
u (the same u on every core) holds rows from the same global rank
block. Each group then gets a compile-time-static column bound L_u
sized from the uniform-position quantile plus margin: columns >= L_u of
the output stay zero via the runtime's pre-zeroed output buffers (both
run_bass_kernel_spmd paths guarantee this), so they are never loaded,
computed, or stored. This cuts HBM traffic to ~64% of the dense kernel.

Rows whose position exceeds their group's static bound (impossible for
uniform positions given the margin, possible for adversarial inputs)
are patched exactly on the host afterwards.

Per tile the compute is one fused in-place DVE op via
scalar_tensor_tensor: tile = (iota < pos_per_partition) * tile.
All DMAs issue on the SP HWDGE ring (measured fastest).
"""

import numpy as np

import concourse.bacc as bacc
import concourse.mybir as mybir
from concourse.tile import TileContext
from concourse.bass_utils import run_bass_kernel_spmd

N, L = 16384, 4096
M = 8               # cores
R = N // M          # 2048 rows per core
P = 128             # partitions
T = R // P          # 16 row-tiles per core
G = 2               # row-tiles fused per DMA group
U = T // G          # DMA groups per core
ROWS_PER_GROUP = G * P          # 256 rows per (core, group)
BLOCK = M * ROWS_PER_GROUP      # 2048 rows of global sorted rank per group

# Static column bound per group u: uniform quantile of the max position in
# global rank block u, plus margin, rounded up to 128.
MARGIN = 320
L_U = [
    min(L, 128 * int(np.ceil((4094.0 * (u + 1) * BLOCK / N + MARGIN) / 128)))
    for u in range(U)
]

_NC = None


def _build(reps=1):
    nc = bacc.Bacc(None, target_bir_lowering=False)
    img = nc.dram_tensor("images", [R, L], mybir.dt.float32, kind="ExternalInput")
    pos = nc.dram_tensor("pos", [P, T], mybir.dt.float32, kind="ExternalInput")
    out = nc.dram_tensor("out", [R, L], mybir.dt.float32, kind="ExternalOutput")

    # group u holds row-tiles t = u*G + g; partition p maps to row (u*G+g)*P + p
    img_t = img.rearrange("(u g p) l -> u p g l", p=P, g=G)
    out_t = out.rearrange("(u g p) l -> u p g l", p=P, g=G)

    with TileContext(nc) as tc:
        with (
            tc.tile_pool(name="const", bufs=1) as cpool,
            tc.tile_pool(name="work", bufs=4) as wpool,
        ):
            pos_sb = cpool.tile([P, T], mybir.dt.float32, tag="pos")
            nc.sync.dma_start(out=pos_sb[:, :], in_=pos[:, :])

            iota_raw = cpool.tile([P, L], mybir.dt.float32, tag="iota_raw")
            nc.gpsimd.iota(
                iota_raw[:, :],
                [[1, L]],
                channel_multiplier=0,
                allow_small_or_imprecise_dtypes=True,
            )
            # Joiner: one DVE op reading iota_raw AND pos_sb, so every later
            # DVE op inherits those dependencies via program order instead of
            # carrying extra semaphore waits. (x + p) - p == x exactly for
            # the small integers involved.
            iota = cpool.tile([P, L], mybir.dt.float32, tag="iota")
            nc.vector.tensor_scalar(
                out=iota[:, :],
                in0=iota_raw[:, :],
                scalar1=pos_sb[:, 0:1],
                scalar2=pos_sb[:, 0:1],
                op0=mybir.AluOpType.add,
                op1=mybir.AluOpType.subtract,
            )

            for _ in range(reps):
                for u in range(U):
                    lu = L_U[u]
                    tile = wpool.tile([P, G, L], mybir.dt.float32, tag="img")
                    nc.sync.dma_start(
                        out=tile[:, :, 0:lu], in_=img_t[u][:, :, 0:lu]
                    )
                    for g in range(G):
                        t = u * G + g
                        # in-place: tile = (iota < pos[:, t]) * tile
                        nc.vector.scalar_tensor_tensor(
                            out=tile[:, g, 0:lu],
                            in0=iota[:, 0:lu],
                            scalar=pos_sb[:, t : t + 1],
                            in1=tile[:, g, 0:lu],
                            op0=mybir.AluOpType.is_lt,
                            op1=mybir.AluOpType.mult,
                        )
                    nc.sync.dma_start(
                        out=out_t[u][:, :, 0:lu], in_=tile[:, :, 0:lu]
                    )
    nc.compile()
    return nc


def _get_nc():
    global _NC
    if _NC is None:
        _NC = _build()
    return _NC


def _sorted_assignment(position):
    """Global sorted order, rank-interleaved: core c, group u gets global
    sorted ranks [u*BLOCK + c*ROWS_PER_GROUP, u*BLOCK + (c+1)*ROWS_PER_GROUP).
    Returns order[core, local_row] = original row index."""
    idx = np.argsort(position, kind="stable")
    order = np.empty((M, R), dtype=np.int64)
    for c in range(M):
        for u in range(U):
            lo = u * BLOCK + c * ROWS_PER_GROUP
            order[c, u * ROWS_PER_GROUP : (u + 1) * ROWS_PER_GROUP] = idx[
                lo : lo + ROWS_PER_GROUP
            ]
    return order


def _make_in_maps(images, position):
    images = np.ascontiguousarray(np.asarray(images), dtype=np.float32)
    pos = np.asarray(position).astype(np.float32)
    order = _sorted_assignment(pos)
    in_maps = []
    for c in range(M):
        rows = order[c]
        img_c = np.ascontiguousarray(images[rows])
        # [T, P] -> [P, T] so column t holds the per-partition positions of tile t
        pos_c = np.ascontiguousarray(pos[rows].reshape(T, P).T)
        in_maps.append({"images": img_c, "pos": pos_c})
    return in_maps, order


def _run(in_maps, **kwargs):
    return run_bass_kernel_spmd(_get_nc(), in_maps, core_ids=list(range(M)), **kwargs)


def kernel(images, position):
    images_np = np.ascontiguousarray(np.asarray(images), dtype=np.float32)
    pos_np = np.asarray(position)
    in_maps, order = _make_in_maps(images_np, pos_np)
    res = _run(in_maps)

    out = np.empty((N, L), dtype=np.float32)
    for c in range(M):
        out[order[c]] = res.results[c]["out"]

    # Exact host fallback for rows whose position exceeds their group's
    # static column bound (never happens for ~uniform positions).
    bounds = np.repeat(np.asarray(L_U, dtype=np.int64), BLOCK)  # per sorted rank
    sorted_pos = pos_np[np.argsort(pos_np, kind="stable")]
    if np.any(sorted_pos > bounds[: len(sorted_pos)]):
        viol_rows = np.concatenate(
            [
                order[c][np.asarray(in_maps[c]["pos"].T.reshape(-1) > np.repeat(L_U, ROWS_PER_GROUP))]
                for c in range(M)
            ]
        )
        ar = np.arange(L)[None, :]
        pv = np.asarray(pos_np)[viol_rows][:, None]
        out[viol_rows] = np.where(ar < pv, images_np[viol_rows], 0.0)
    return out
